# revision 1
# baseline (speedup 1.0000x reference)
"""AffineCoupling TRN2 kernel.

Computes, for z [4_000_000, 16] fp32:
    zl = z[:, :8]; zr = z[:, 8:]
    log_s = MLP_logs(zl); b = MLP_b(zl)        (5 layers, LeakyReLU(0.01) between)
    out = concat([zl, exp(log_s) * zr + b], axis=1)

Strategy (pure data parallel over 8 NeuronCores):
 - Each core gets a 507,904-row slice (slices overlap slightly to cover 4M).
 - On-chip layout: "nat" tile [128, 2048] holds 16,384 rows, 8 rows/partition
   per 128-col sub-tile: nat[p, s*128 + g*16 + f] = z[base + s*1024 + p*8 + g, f].
   HBM reads/writes are 512B-contiguous per partition.
 - PE transposes each [128,128] sub-tile to feature-major (X0[g*16+f, col]).
 - MLP = 5 bf16 matmuls per 4096-row chunk against block-diagonal augmented
   weights (both branches fused; fp32->bf16 casts ride the PSUM->SBUF copies);
   LeakyReLU = ACT Prelu(alpha=0.01) reading the fp32 PSUM, writing bf16,
   with the layer bias applied via the activation's per-partition bias operand.
 - L5 writes fp32 [log_s (parts 0:64, g*8+o) ; b (parts 64:128)]; Exp and a
   bias-add assemble eb = [e; b] in fp32, which is PE-transposed back to
   batch-major and combined with zr in the nat tile in place: yr = e * zr + b.
 - The nat tile (zl untouched, zr overwritten with yr) is DMA'd back out.
"""
import os
import sys

sys.path.insert(0, "/opt/trn_rl_repo")
if "/root/.axon_site/_ro/trn_rl_repo" not in sys.path:
    sys.path.append("/root/.axon_site/_ro/trn_rl_repo")

import numpy as np

import concourse.bacc as bacc
import concourse.bass as bass
import concourse.tile as tile
from concourse import mybir
from concourse.bass import _add_dep_helper
from concourse.bass_utils import run_bass_kernel_spmd

FP = mybir.dt.float32
BF = mybir.dt.bfloat16

N_CORES = 8
BATCH = 4_000_000
ROWS_PER_MACRO = 16_384            # [128, 2048] nat tile
MACROS = 31
R = ROWS_PER_MACRO * MACROS        # 507,904 rows per core
PAD_ROWS = ROWS_PER_MACRO          # guard band: writes never touch tensor tail
SUBTILES = 16                      # per macro, 1024 rows each
CHUNKS = 4                         # per macro, 4096 rows each (4 sub-tiles)
NAT_BUFS = 3

STEP = 498_688
# cores 0..6 tile forward; the last core is pinned to cover the batch tail
STARTS = [c * STEP for c in range(N_CORES - 1)] + [BATCH - R]

C_BIAS = 128                       # fp32 consts: identity + 5 bias columns
C_TOTAL = 133

LAST_RESULTS = None


def _build_consts(ws_logs, bs_logs, ws_b, bs_b):
    import ml_dtypes

    ws_logs = [np.asarray(w, np.float32) for w in ws_logs]
    bs_logs = [np.asarray(b, np.float32) for b in bs_logs]
    ws_b = [np.asarray(w, np.float32) for w in ws_b]
    bs_b = [np.asarray(b, np.float32) for b in bs_b]

    consts = np.zeros((128, C_TOTAL), np.float32)
    consts[:, 0:128] = np.eye(128, dtype=np.float32)
    for k in range(4):
        cat = np.concatenate([bs_logs[k], bs_b[k]])    # [16]
        consts[:, C_BIAS + k] = np.tile(cat, 8)
    consts[:, C_BIAS + 4] = np.concatenate(
        [np.tile(bs_logs[4], 8), np.tile(bs_b[4], 8)]
    )

    # bf16 stationary matrices, lhsT k at cols [k*128, (k+1)*128)
    wmat = np.zeros((128, 5 * 128), np.float32)
    # L1: input rows g*16+i (i<8: zl features), output cols g*16+o_cat
    w1cat = np.vstack([ws_logs[0], ws_b[0]])           # [16, 8]
    for g in range(8):
        wmat[g * 16:g * 16 + 8, g * 16:(g + 1) * 16] = w1cat.T
    for k in (1, 2, 3):
        wk = np.zeros((16, 16), np.float32)
        wk[0:8, 0:8] = ws_logs[k]
        wk[8:16, 8:16] = ws_b[k]
        for g in range(8):
            wmat[g * 16:(g + 1) * 16, k * 128 + g * 16:k * 128 + (g + 1) * 16] = wk.T
    for g in range(8):
        wmat[g * 16:g * 16 + 8, 4 * 128 + g * 8:4 * 128 + (g + 1) * 8] = ws_logs[4].T
        wmat[g * 16 + 8:(g + 1) * 16,
             4 * 128 + 64 + g * 8:4 * 128 + 64 + (g + 1) * 8] = ws_b[4].T
    wmat = np.concatenate([wmat, np.eye(128, dtype=np.float32)], axis=1)
    wmat_bf = wmat.astype(ml_dtypes.bfloat16)
    return consts, wmat_bf


def _free_ap(t, offset, dims):
    """AP over tile t with the tile's partition dim, explicit free dims
    [[step, count], ...] and an element offset into the free space."""
    return bass.AP(tensor=t.tensor, offset=t.offset + offset, ap=[t.ap[0]] + dims)


def _build_nc():
    nc = bacc.Bacc()
    z_d = nc.declare_dram_parameter("z", [R + PAD_ROWS, 16], FP, isOutput=False)
    c_d = nc.declare_dram_parameter("consts", [128, C_TOTAL], FP, isOutput=False)
    w_d = nc.declare_dram_parameter("wmat", [128, 6 * 128], BF, isOutput=False)
    o_d = nc.declare_dram_parameter("out", [R + PAD_ROWS, 16], FP, isOutput=True)

    with tile.TileContext(nc) as tc:
        with (
            tc.tile_pool(name="consts", bufs=1) as cp,
            tc.tile_pool(name="nat", bufs=4) as natp,
            tc.tile_pool(name="sb", bufs=4) as sbp,
            tc.tile_pool(name="ps", bufs=2, space="PSUM") as psp,
            tc.tile_pool(name="hps", bufs=3, space="PSUM") as hpsp,
        ):
            consts = cp.tile([128, C_TOTAL], FP)
            nc.sync.dma_start(out=consts, in_=c_d[:, :])
            wmat = cp.tile([128, 6 * 128], BF)
            nc.sync.dma_start(out=wmat, in_=w_d[:, :])
            ident = consts[:, 0:128]
            identbf = wmat[:, 5 * 128:6 * 128]
            lhsT = [wmat[:, k * 128:(k + 1) * 128] for k in range(5)]
            biases = [consts[:, C_BIAS + k:C_BIAS + k + 1] for k in range(5)]

            # warm up each engine's vector clock on the const DMAs
            wu_ps = psp.tile([128, 128], FP, tag="tp")
            nc.tensor.matmul(wu_ps, ident, ident, start=True, stop=True)
            wu_ps2 = hpsp.tile([64, 64], FP, tag="hp")
            nc.tensor.matmul(wu_ps2, lhsT[0][:, 0:64], wmat[:, 0:64],
                             start=True, stop=True)
            wu1 = sbp.tile([128, 1], FP, tag="wu")
            nc.scalar.copy(out=wu1, in_=biases[0])
            wu2 = sbp.tile([128, 1], FP, tag="wu")
            nc.vector.tensor_copy(out=wu2, in_=biases[0])
            wu3 = sbp.tile([128, 1], FP, tag="wu")
            nc.gpsimd.tensor_copy(out=wu3, in_=biases[0])

            tail_dmas = []
            for m in range(MACROS):
                r0 = m * ROWS_PER_MACRO
                nat = natp.tile([128, 2048], FP, tag="nat")
                nc.sync.dma_start(
                    out=nat.rearrange("p (s g f) -> p s g f", s=SUBTILES, g=8, f=16),
                    in_=z_d[r0:r0 + ROWS_PER_MACRO, :].rearrange(
                        "(s p g) f -> p s g f", s=SUBTILES, p=128, g=8
                    ),
                )

                natbfs = []
                for k in range(CHUNKS):
                    natbf = sbp.tile([128, 512], BF, tag="natbf", bufs=12)
                    nc.gpsimd.tensor_copy(
                        out=natbf, in_=nat[:, k * 512:(k + 1) * 512])
                    natbfs.append(natbf)

                for j in range(CHUNKS // 2):          # chunk pairs
                    x0s = []
                    for c in range(2):                # per-chunk transposes + cast
                        k = 2 * j + c
                        x0ps = psp.tile([128, 512], FP, tag="tp")
                        natbf = natbfs[k]
                        for t in range(4):
                            nc.tensor.matmul(
                                x0ps[:, t * 128:(t + 1) * 128],
                                natbf[:, t * 128:(t + 1) * 128],
                                identbf,
                                start=True, stop=True,
                            )
                        x0 = sbp.tile([128, 512], BF, tag="x0", bufs=8)
                        nc.vector.tensor_copy(out=x0, in_=x0ps)
                        x0s.append(x0)

                    # ---- MLP: bf16 matmul pairs -> [128,1024] Prelu
                    h = x0s
                    for layer in range(4):
                        hp = hpsp.tile([128, 1024], FP, tag="hp")
                        for c in range(2):
                            nc.tensor.matmul(hp[:, c * 512:(c + 1) * 512],
                                             lhsT[layer], h[c],
                                             start=True, stop=True)
                        hb = sbp.tile([128, 1024], BF, tag="h", bufs=8)
                        nc.scalar.activation(
                            out=hb, in_=hp,
                            func=mybir.ActivationFunctionType.Prelu,
                            bias=biases[layer], scale=1.0, alpha=0.01,
                        )
                        h = [hb[:, 0:512], hb[:, 512:1024]]
                    hp5 = hpsp.tile([128, 1024], FP, tag="hp")
                    for c in range(2):
                        nc.tensor.matmul(hp5[:, c * 512:(c + 1) * 512],
                                         lhsT[4], h[c], start=True, stop=True)

                    # ---- eb = [exp(log_s + b5L) ; b + b5R]   (fp32, both chunks)
                    eb = sbp.tile([128, 1024], FP, tag="eb", bufs=6)
                    nc.scalar.activation(
                        out=eb[0:64, :], in_=hp5[0:64, :],
                        func=mybir.ActivationFunctionType.Exp,
                        bias=biases[4][0:64, :], scale=1.0,
                    )
                    nc.vector.tensor_scalar_add(
                        out=eb[64:128, :], in0=hp5[64:128, :],
                        scalar1=biases[4][64:128, :],
                    )

                    # ---- per chunk: transpose back + yr = e*zr + b in place
                    for c in range(2):
                        k = 2 * j + c
                        ebT = psp.tile([128, 512], FP, tag="tp")
                        for t in range(4):
                            nc.tensor.transpose(
                                ebT[:, t * 128:(t + 1) * 128],
                                eb[:, c * 512 + t * 128:c * 512 + (t + 1) * 128],
                                ident,
                            )
                        e_ap = _free_ap(ebT, 0, [[128, 4], [8, 8], [1, 8]])
                        b_ap = _free_ap(ebT, 64, [[128, 4], [8, 8], [1, 8]])
                        zr_ap = _free_ap(nat, k * 512 + 8,
                                         [[128, 4], [16, 8], [1, 8]])
                        tmp = sbp.tile([128, 256], FP, tag="tmp", bufs=8)
                        tmp_ap = _free_ap(tmp, 0, [[64, 4], [8, 8], [1, 8]])
                        nc.vector.tensor_mul(out=tmp_ap, in0=e_ap, in1=zr_ap)
                        nc.vector.tensor_add(out=zr_ap, in0=tmp_ap, in1=b_ap)

                out_dma = nc.sync.dma_start(
                    out=o_d[r0:r0 + ROWS_PER_MACRO, :].rearrange(
                        "(s p g) f -> p s g f", s=SUBTILES, p=128, g=8
                    ),
                    in_=nat.rearrange("p (s g f) -> p s g f", s=SUBTILES, g=8, f=16),
                )
                if m >= MACROS - NAT_BUFS:
                    tail_dmas.append(out_dma)

            flush = sbp.tile([128, 1], FP, tag="wu")
            fl = nc.vector.tensor_copy(out=flush, in_=biases[0])
            for dma in tail_dmas:
                _add_dep_helper(fl.ins, dma.ins, sync=True,
                                reason="drain tail out-DMAs before kernel end")

    nc.finalize()
    return nc


_NC_CACHE = None


def kernel(z, ws_logs, bs_logs, ws_b, bs_b):
    global _NC_CACHE, LAST_RESULTS
    z = np.asarray(z, np.float32)
    assert z.shape == (BATCH, 16)
    consts, wmat_bf = _build_consts(ws_logs, bs_logs, ws_b, bs_b)

    if _NC_CACHE is None:
        _NC_CACHE = _build_nc()
    nc = _NC_CACHE

    in_maps = []
    for s in STARTS:
        zp = np.zeros((R + PAD_ROWS, 16), np.float32)
        zp[:R] = z[s:s + R]
        in_maps.append({"z": zp, "consts": consts, "wmat": wmat_bf})
    trace = bool(os.environ.get("AFFINE_TRACE"))
    res = run_bass_kernel_spmd(nc, in_maps, core_ids=list(range(N_CORES)), trace=trace)
    LAST_RESULTS = res

    out = np.empty((BATCH, 16), np.float32)
    for c in range(N_CORES):
        out[STARTS[c]:STARTS[c] + R] = res.results[c]["out"][:R]
    return out



# revision 9
# speedup vs baseline: 2.1048x; 2.1048x over previous
"""AffineCoupling TRN2 kernel (v2).

Computes, for z [4_000_000, 16] fp32:
    zl = z[:, :8]; zr = z[:, 8:]
    log_s = MLP_logs(zl); b = MLP_b(zl)        (5 layers, LeakyReLU(0.01) between)
    out = concat([zl, exp(log_s) * zr + b], axis=1)

Strategy (pure data parallel over 8 NeuronCores):
 - Each core gets a 507,904-row slice (slices overlap slightly to cover 4M).
 - nat tile [128, 2048] fp32 holds 16,384 rows: nat[p, s*128+g*16+f] =
   z[r0 + s*1024 + p*8 + g, f]; HBM I/O is 512B-contiguous per partition.
 - Feature-major transform packs 16 rows/column: partition q = g*16+s''*8+f
   (s'' picks subtile u vs u+8), column u*128+p.  Transpose-in = 8 regular
   matmuls with strided fp32 nat slices as lhsT against an fp32 identity
   (NOT transpose-mode, which runs slow and starves the PE clock gate).
 - The two 5-layer 8->8 MLP chains (log_s / b) run branch-split as bf16
   matmuls, 2x N=512 per layer per branch, against block-diagonal lhsT
   (16 blocks of the 8x8 weights along q).
 - LeakyReLU instances are spread across engines: ACT Prelu (bias operand),
   DVE/GpSimd 2-op form: t=(u+bias)*0.01 then max(u+bias, t) via
   scalar_tensor_tensor.
 - e = Exp(log_s + b5e) on ACT -> bf16; b-cast via DVE tensor_scalar_add;
   both transpose back with 16 regular bf16 matmuls; yr = e*zr + b via two
   DVE tensor_tensor ops per half-macro, writing zr in place in nat.
"""
import os
import sys

sys.path.insert(0, "/opt/trn_rl_repo")
if "/root/.axon_site/_ro/trn_rl_repo" not in sys.path:
    sys.path.append("/root/.axon_site/_ro/trn_rl_repo")

import numpy as np

import concourse.bacc as bacc
import concourse.bass as bass
import concourse.tile as tile
from concourse import mybir
from concourse.bass import _add_dep_helper
from concourse.bass_utils import run_bass_kernel_spmd

FP = mybir.dt.float32
BF = mybir.dt.bfloat16

N_CORES = 8
BATCH = 4_000_000
ROWS_PER_MACRO = 16_384            # [128, 2048] nat tile
MACROS = 31
R = ROWS_PER_MACRO * MACROS        # 507,904 rows per core
PAD_ROWS = ROWS_PER_MACRO          # guard band: writes never touch tensor tail
SUBTILES = 16
NAT_BUFS = 3

STEP = 498_688
STARTS = [c * STEP for c in range(N_CORES - 1)] + [BATCH - R]

# consts fp32 layout: identity fp32 (128 cols) + 10 bias columns
C_BIAS = 128
C_TOTAL = 138
# wmat bf16 layout: identbf (128 cols) + 10 lhsT blocks (k=1..5) x (s,b)
W_BLOCKS = 11

LAST_RESULTS = None

ALPHA = 0.01


def _build_consts(ws_logs, bs_logs, ws_b, bs_b):
    import ml_dtypes

    ws_logs = [np.asarray(w, np.float32) for w in ws_logs]
    bs_logs = [np.asarray(b, np.float32) for b in bs_logs]
    ws_b = [np.asarray(w, np.float32) for w in ws_b]
    bs_b = [np.asarray(b, np.float32) for b in bs_b]

    consts = np.zeros((128, C_TOTAL), np.float32)
    consts[:, 0:128] = np.eye(128, dtype=np.float32)
    # bias columns: index = low 3 bits of partition q -> tile(bias, 16)
    for k in range(4):
        consts[:, C_BIAS + k] = np.tile(bs_logs[k], 16)
        consts[:, C_BIAS + 4 + k] = np.tile(bs_b[k], 16)
    consts[:, C_BIAS + 8] = np.tile(bs_logs[4], 16)   # exp bias
    consts[:, C_BIAS + 9] = np.tile(bs_b[4], 16)      # b-cast bias

    wmat = np.zeros((128, W_BLOCKS * 128), np.float32)
    wmat[:, 0:128] = np.eye(128, dtype=np.float32)
    # lhsT blocks: block index 1 + (k-1)*2 + (0 for s, 1 for b)
    for k in range(5):
        for bi, ws in ((0, ws_logs[k]), (1, ws_b[k])):
            blk = 1 + k * 2 + bi
            lhsT = np.zeros((128, 128), np.float32)
            for t in range(16):          # t = g*2 + s''; base = t*8 in q-order
                base = t * 8
                lhsT[base:base + 8, base:base + 8] = ws.T
            wmat[:, blk * 128:(blk + 1) * 128] = lhsT
    wmat_bf = wmat.astype(ml_dtypes.bfloat16)
    return consts, wmat_bf


def _ap(t, offset, dims):
    """AP over tile t keeping its partition dim, explicit free dims
    [[step, count], ...] and an element offset into the free space."""
    return bass.AP(tensor=t.tensor, offset=t.offset + offset, ap=[t.ap[0]] + dims)


def _build_nc():
    nc = bacc.Bacc()
    z_d = nc.declare_dram_parameter("z", [R + PAD_ROWS, 16], FP, isOutput=False)
    c_d = nc.declare_dram_parameter("consts", [128, C_TOTAL], FP, isOutput=False)
    w_d = nc.declare_dram_parameter("wmat", [128, W_BLOCKS * 128], BF, isOutput=False)
    o_d = nc.declare_dram_parameter("out", [R + PAD_ROWS, 16], FP, isOutput=True)

    with tile.TileContext(nc) as tc:
        with (
            tc.tile_pool(name="consts", bufs=1) as cp,
            tc.tile_pool(name="nat", bufs=NAT_BUFS) as natp,
            tc.tile_pool(name="sb", bufs=2) as sbp,
            tc.tile_pool(name="pshp", bufs=3, space="PSUM") as pshp,
            tc.tile_pool(name="pstb", bufs=2, space="PSUM") as pstb,
        ):
            consts = cp.tile([128, C_TOTAL], FP)
            nc.sync.dma_start(out=consts, in_=c_d[:, :])
            wmat = cp.tile([128, W_BLOCKS * 128], BF)
            nc.sync.dma_start(out=wmat, in_=w_d[:, :])
            ident = consts[:, 0:128]
            identbf = wmat[:, 0:128]
            lhsT = {}
            for k in range(5):
                for bi, beta in ((0, "s"), (1, "b")):
                    blk = 1 + k * 2 + bi
                    lhsT[(k, beta)] = wmat[:, blk * 128:(blk + 1) * 128]
            bias = {}
            for k in range(4):
                bias[(k, "s")] = consts[:, C_BIAS + k:C_BIAS + k + 1]
                bias[(k, "b")] = consts[:, C_BIAS + 4 + k:C_BIAS + 5 + k]
            bias_e = consts[:, C_BIAS + 8:C_BIAS + 9]
            bias_b5 = consts[:, C_BIAS + 9:C_BIAS + 10]

            # warm up engines
            wu_ps = pstb.tile([128, 512], FP, tag="tb")
            nc.tensor.matmul(wu_ps[:, 0:128], ident, ident, start=True, stop=True)
            wu1 = sbp.tile([128, 1], FP, tag="wu", bufs=2)
            nc.scalar.copy(out=wu1, in_=bias_e)
            wu2 = sbp.tile([128, 1], FP, tag="wu")
            nc.vector.tensor_copy(out=wu2, in_=bias_e)
            wu3 = sbp.tile([128, 1], FP, tag="wu")
            nc.gpsimd.tensor_copy(out=wu3, in_=bias_e)

            # LeakyReLU engine assignment per (layer k=0..3, branch).
            # GpSimd has no PSUM port, so only ACT (1-op Prelu w/ bias) and
            # DVE (2-op: bias-add cast to bf16, then max(u, 0.01u) via
            # scalar_tensor_tensor in all-bf16 SBUF for the 2x DVE mode).
            # "split" runs the first half [*, :512] on ACT, second on DVE.
            ENG = {
                (0, "s"): "act", (0, "b"): "act",
                (1, "s"): "act", (1, "b"): "act",
                (2, "s"): "act", (2, "b"): "act",
                (3, "s"): "split", (3, "b"): "dve",
            }

            def leaky_act(hp, bias_ap, hout):
                nc.scalar.activation(
                    out=hout, in_=hp,
                    func=mybir.ActivationFunctionType.Prelu,
                    bias=bias_ap, scale=1.0, alpha=ALPHA,
                )

            def leaky_dve(hp, bias_ap, hout, tscratch):
                nc.vector.tensor_scalar_add(
                    out=tscratch, in0=hp, scalar1=bias_ap)
                nc.vector.scalar_tensor_tensor(
                    out=hout, in0=tscratch, scalar=ALPHA, in1=tscratch,
                    op0=mybir.AluOpType.mult, op1=mybir.AluOpType.max,
                )

            def leaky(eng, hp, bias_ap, hout, tscratch):
                if eng == "act":
                    leaky_act(hp, bias_ap, hout)
                elif eng == "dve":
                    leaky_dve(hp, bias_ap, hout, tscratch)
                else:
                    leaky_act(hp[:, 0:512], bias_ap, hout[:, 0:512])
                    leaky_dve(hp[:, 512:1024], bias_ap, hout[:, 512:1024],
                              tscratch[:, 512:1024])

            tail_dmas = []
            for m in range(MACROS):
                r0 = m * ROWS_PER_MACRO
                nat = natp.tile([128, 2048], FP, tag="nat")
                nc.sync.dma_start(
                    out=nat.rearrange("p (s g f) -> p s g f", s=SUBTILES, g=8, f=16),
                    in_=z_d[r0:r0 + ROWS_PER_MACRO, :].rearrange(
                        "(s p g) f -> p s g f", s=SUBTILES, p=128, g=8
                    ),
                )

                # ---- zl gather-cast to bf16 (GpSimd, SBUF->SBUF):
                # natzl[p, u*128 + g*16 + s''*8 + f] = nat[p, (u+8s'')*128+g*16+f]
                natzl = sbp.tile([128, 1024], BF, tag="nzl", bufs=2)
                for sp in range(2):
                    nc.gpsimd.tensor_copy(
                        out=_ap(natzl, sp * 8, [[128, 8], [16, 8], [1, 8]]),
                        in_=_ap(nat, sp * 1024, [[128, 8], [16, 8], [1, 8]]),
                    )

                # ---- transpose-in: -> feature-major x0 [128, 1024] bf16
                # q = g*16 + s''*8 + f ; col = u*128 + p ; s = u + s''*8
                x0ps = pshp.tile([128, 1024], FP, tag="hp")
                for u in range(8):
                    nc.tensor.matmul(
                        x0ps[:, u * 128:(u + 1) * 128],
                        natzl[:, u * 128:(u + 1) * 128],
                        identbf, start=True, stop=True,
                    )
                x0 = sbp.tile([128, 1024], BF, tag="x0", bufs=2)
                nc.vector.tensor_copy(out=x0, in_=x0ps)

                # ---- MLP chains (branch-split, interleaved for overlap)
                cur = {"s": x0, "b": x0}
                for k in range(4):
                    hps = {}
                    for beta in ("s", "b"):
                        hp = pshp.tile([128, 1024], FP, tag="hp")
                        for c in range(2):
                            nc.tensor.matmul(
                                hp[:, c * 512:(c + 1) * 512],
                                lhsT[(k, beta)],
                                cur[beta][:, c * 512:(c + 1) * 512],
                                start=True, stop=True,
                            )
                        hps[beta] = hp
                    for beta in ("s", "b"):
                        eng = ENG[(k, beta)]
                        hout = sbp.tile([128, 1024], BF, tag="h", bufs=6)
                        tscratch = None
                        if eng != "act":
                            tscratch = sbp.tile([128, 1024], BF, tag="t", bufs=2)
                        leaky(eng, hps[beta], bias[(k, beta)], hout, tscratch)
                        cur[beta] = hout

                # ---- L5 + e/b casts
                hp5 = {}
                for beta in ("s", "b"):
                    hp = pshp.tile([128, 1024], FP, tag="hp")
                    for c in range(2):
                        nc.tensor.matmul(
                            hp[:, c * 512:(c + 1) * 512],
                            lhsT[(4, beta)],
                            cur[beta][:, c * 512:(c + 1) * 512],
                            start=True, stop=True,
                        )
                    hp5[beta] = hp
                ebf_e = sbp.tile([128, 1024], BF, tag="eb", bufs=4)
                nc.scalar.activation(
                    out=ebf_e, in_=hp5["s"],
                    func=mybir.ActivationFunctionType.Exp,
                    bias=bias_e, scale=1.0,
                )
                ebf_b = sbp.tile([128, 1024], BF, tag="eb", bufs=4)
                nc.vector.tensor_scalar_add(
                    out=ebf_b, in0=hp5["b"], scalar1=bias_b5)

                # ---- transpose-back + combine, per half-macro
                # eT[p, j*128 + q] ; q = g*16+s''*8+o
                # nat zr col = u*128 + s''*1024 + g*16 + 8 + o  (u = h*4+j)
                for h in range(2):
                    eT = pstb.tile([128, 512], FP, tag="tb")
                    bT = pstb.tile([128, 512], FP, tag="tb")
                    for j in range(4):
                        u = h * 4 + j
                        nc.tensor.matmul(
                            eT[:, j * 128:(j + 1) * 128],
                            ebf_e[:, u * 128:(u + 1) * 128],
                            identbf, start=True, stop=True,
                        )
                        nc.tensor.matmul(
                            bT[:, j * 128:(j + 1) * 128],
                            ebf_b[:, u * 128:(u + 1) * 128],
                            identbf, start=True, stop=True,
                        )
                    # (ug, s'', o) dims: eT side steps (16, 8, 1);
                    # nat side steps (16, 1024, 1), offset h*512 + 8
                    et_ap = _ap(eT, 0, [[16, 32], [8, 2], [1, 8]])
                    bt_ap = _ap(bT, 0, [[16, 32], [8, 2], [1, 8]])
                    zr_ap = _ap(nat, h * 512 + 8, [[16, 32], [1024, 2], [1, 8]])
                    tmp = sbp.tile([128, 512], FP, tag="tmp", bufs=4)
                    tmp_ap = _ap(tmp, 0, [[16, 32], [8, 2], [1, 8]])
                    nc.vector.tensor_mul(out=tmp_ap, in0=et_ap, in1=zr_ap)
                    nc.vector.tensor_add(out=zr_ap, in0=tmp_ap, in1=bt_ap)

                out_dma = nc.sync.dma_start(
                    out=o_d[r0:r0 + ROWS_PER_MACRO, :].rearrange(
                        "(s p g) f -> p s g f", s=SUBTILES, p=128, g=8
                    ),
                    in_=nat.rearrange("p (s g f) -> p s g f", s=SUBTILES, g=8, f=16),
                )
                if m >= MACROS - NAT_BUFS:
                    tail_dmas.append(out_dma)

            flush = sbp.tile([128, 1], FP, tag="wu")
            fl = nc.vector.tensor_copy(out=flush, in_=bias_e)
            for dma in tail_dmas:
                _add_dep_helper(fl.ins, dma.ins, sync=True,
                                reason="drain tail out-DMAs before kernel end")

    nc.finalize()
    return nc


_NC_CACHE = None


def kernel(z, ws_logs, bs_logs, ws_b, bs_b):
    global _NC_CACHE, LAST_RESULTS
    z = np.asarray(z, np.float32)
    assert z.shape == (BATCH, 16)
    consts, wmat_bf = _build_consts(ws_logs, bs_logs, ws_b, bs_b)

    if _NC_CACHE is None:
        _NC_CACHE = _build_nc()
    nc = _NC_CACHE

    in_maps = []
    for s in STARTS:
        zp = np.zeros((R + PAD_ROWS, 16), np.float32)
        zp[:R] = z[s:s + R]
        in_maps.append({"z": zp, "consts": consts, "wmat": wmat_bf})
    trace = bool(os.environ.get("AFFINE_TRACE"))
    res = run_bass_kernel_spmd(nc, in_maps, core_ids=list(range(N_CORES)), trace=trace)
    LAST_RESULTS = res

    out = np.empty((BATCH, 16), np.float32)
    for c in range(N_CORES):
        out[STARTS[c]:STARTS[c] + R] = res.results[c]["out"][:R]
    return out
